# revision 64
# baseline (speedup 1.0000x reference)
"""Multi-head causal attention + output projection on 8 Trainium2 cores.

Problem: B=4, T=2048, H=16, DQK=DV=64, E=1024, causal mask, fp32.

Sharding: core c -> batch b = c//2, head-group g = c%2 (8 heads each).
Each core computes full causal attention for its 8 heads and a partial
output projection (its heads' rows of W_o). Host sums the two partial
projections per batch and adds b_o.

Device algorithm (per head pair hp, q-block of 512, k-tiles of 128):
  scores^T(k,q) = K_h Q_h^T            f32r matmul, k on partitions
  causal diag tiles: PE adds a -1e30 strict-lower-tri bias matmul
                     (lhsT=I, rhs=Lneg) into the scores PSUM group
  at = exp(scores^T/8) -> bf16         (ACT; no max-subtraction)
  PV flipped: per q-chunk of 128, stationary = at[k,qchunk] (bf16),
    moving = [V_h | 1] (bf16, 65 cols) -> ctx_nat[q,65] PSUM accum
    over k-tiles; col 64 = softmax denominators.
  norm+transpose in one matmul: stationary = raw ctx (bf16), moving =
    diag(1/sums) (bf16) -> ctxT[d,q] normalized, evac to SBUF bf16
  out(q,E) = ctxT.T @ W_o              (bf16 operands, f32 PSUM)
"""

import numpy as np
import ml_dtypes

import concourse.bass as bass
import concourse.mybir as mybir
import concourse.tile as tile
from concourse import bacc
from concourse.bass_utils import run_bass_kernel_spmd

B, T, H, D, E = 4, 2048, 16, 64, 1024
HLOC = 8            # heads per core
NCORES = 8
TQ = 512            # q-block size
TK = 128            # k-tile size
NQB = T // TQ       # 4
NHP = HLOC // 2     # 4 head pairs
NKT = T // TK       # 16 k-tiles total
SCALE = 1.0 / np.sqrt(D)

F32 = mybir.dt.float32
F32R = mybir.dt.float32r
BF16 = mybir.dt.bfloat16


def _build_nc():
    nc = bacc.Bacc("TRN2", target_bir_lowering=False, debug=False,
                   num_devices=NCORES, name="mha")
    qt_d = nc.dram_tensor("qt", [HLOC * D, T], F32R, kind="ExternalInput")
    kt_d = nc.dram_tensor("kt", [HLOC * D, T], F32R, kind="ExternalInput")
    vo_d = nc.dram_tensor("vo", [128, NKT, HLOC, 65], BF16, kind="ExternalInput")
    wo_d = nc.dram_tensor("wo", [HLOC * D, E], BF16, kind="ExternalInput")
    lneg_d = nc.dram_tensor("lneg", [TK, TK], BF16, kind="ExternalInput")
    i128_d = nc.dram_tensor("i128", [128, 128], BF16, kind="ExternalInput")
    # boot tensor: first kt tile + first qt block packed side by side so the
    # critical first QK waits on a single DMA issue
    boot_d = nc.dram_tensor("boot", [128, TK + TQ], F32R, kind="ExternalInput")
    # partial projections stored bf16: halves store DMA traffic; the host
    # sums the two partials per batch in f32 (adds ~0.006 abs err vs the
    # 0.086 budget)
    out_d = nc.dram_tensor("out", [T, E], BF16, kind="ExternalOutput")

    EXP = mybir.ActivationFunctionType.Exp

    with tile.TileContext(nc) as tc:
        with (
            tc.tile_pool(name="const", bufs=1) as const_pool,
            tc.tile_pool(name="qkt", bufs=1) as qkt_pool,
            tc.tile_pool(name="vsb", bufs=1) as v_pool,
            tc.tile_pool(name="ctxT", bufs=1) as ctxT_pool,
            tc.tile_pool(name="attn", bufs=4) as at_pool,
            tc.tile_pool(name="cx", bufs=2) as ctx_pool,
            tc.tile_pool(name="rc", bufs=2) as rc_pool,
            tc.tile_pool(name="diag", bufs=10) as diag_pool,
            tc.tile_pool(name="outsb", bufs=3) as out_pool,
            tc.tile_pool(name="scores", bufs=2, space="PSUM") as scores_pool,
            tc.tile_pool(name="accu", bufs=1, space="PSUM") as accu_pool,
            tc.tile_pool(name="aux", bufs=2, space="PSUM") as aux_pool,
        ):
            ctxT = ctxT_pool.tile([128, NHP, T], BF16)

            kt_sbs, qt_sbs = [], []
            for hp in range(NHP):
                kt_sb = qkt_pool.tile([128, T], F32R, tag=f"kt{hp}", name="kt_sb")
                qt_sb = qkt_pool.tile([128, T], F32R, tag=f"qt{hp}", name="qt_sb")
                kt_sbs.append(kt_sb)
                qt_sbs.append(qt_sb)
            v_sb = v_pool.tile([128, NKT, HLOC, 65], BF16)

            # loads, first-needed first. First step is (qb=3, hp=0, kk=0):
            # kt0 cols 0:128, qt0 top block, v k-tiles 0:2; then progressively
            # the rest of hp0 (qb=3 consumes all 16 k-tiles), then hp1-3.
            # block schedule: open on (qb3,hp0),(qb3,hp1) whose 16-ktile
            # spans cover the input DMA; end on (qb3,hp2),(qb3,hp3) whose
            # long exps absorb backlog and cover the per-chunk endgame
            BLOCKS = ([(3, 0), (3, 1)]
                      + [(0, hp) for hp in range(NHP)]
                      + [(2, hp) for hp in range(NHP)]
                      + [(1, hp) for hp in range(NHP)]
                      + [(3, 2), (3, 3)])

            def kt_load(hp, c0, c1):
                hsl = slice(hp * 128, (hp + 1) * 128)
                nc.sync.dma_start(kt_sbs[hp][:, c0:c1], kt_d[hsl, c0:c1])

            def qt_load(hp, qb):
                hsl = slice(hp * 128, (hp + 1) * 128)
                csl = slice(qb * TQ, (qb + 1) * TQ)
                nc.sync.dma_start(qt_sbs[hp][:, csl], qt_d[hsl, csl])

            boot_sb = const_pool.tile([128, TK + TQ], F32R)
            nc.sync.dma_start(boot_sb[:], boot_d[:])
            lneg_sb = const_pool.tile([TK, TK], BF16)
            nc.sync.dma_start(lneg_sb[:], lneg_d[:])
            i128_sb = const_pool.tile([128, 128], BF16)
            nc.sync.dma_start(i128_sb[:], i128_d[:])
            kt_load(0, TK, 512)
            # warm the PE p-state while the first loads land: harmless
            # matmuls on the const tiles into an aux slot nobody reads
            warm = aux_pool.tile([128, 512], F32, tag="aux", name="warm")
            for w in range(10):
                nc.tensor.matmul(warm[:, 0:128], lhsT=i128_sb[:],
                                 rhs=lneg_sb[:], start=True, stop=True,
                                 skip_group_check=True)
            nc.sync.dma_start(v_sb[:, 0:2], vo_d[:, 0:2])
            nc.sync.dma_start(v_sb[:, 2:4], vo_d[:, 2:4])
            kt_load(0, 0, TK)  # off critical path: later blocks' kk=0 use it
            for ch in range(1, 4):
                kt_load(0, ch * 512, (ch + 1) * 512)
            kt_load(1, 0, 512)
            qt_load(1, 3)
            for ch in range(2, 8):
                nc.sync.dma_start(v_sb[:, 2 * ch:2 * ch + 2],
                                  vo_d[:, 2 * ch:2 * ch + 2])
            for ch in range(1, 4):
                kt_load(1, ch * 512, (ch + 1) * 512)
            qt_load(0, 0)
            qt_load(1, 0)
            for hp in (2, 3):
                for ch in range(4):
                    kt_load(hp, ch * 512, (ch + 1) * 512)
                qt_load(hp, 0)
            for qb in (2, 1, 3):
                for hp in range(NHP):
                    if (qb, hp) in ((3, 0), (3, 1)):
                        continue
                    qt_load(hp, qb)
            wo_sb = const_pool.tile([128, 4, E], BF16)
            nc.sync.dma_start(wo_sb[:], wo_d.rearrange("(n p) e -> p n e", p=128))

            COPY = mybir.ActivationFunctionType.Copy

            def proj_quanta(qt_, tail=False, act_evac=False):
                """Yield the projection of q-tile qt_ as a list of small
                emission closures (~2 matmuls each) so background PE work
                never delays the next QK by more than ~0.4us. tail=True
                (attention fully drained) borrows the idle scores banks for
                a deeper PSUM rotation; act_evac evacuates on the (idle)
                ACT engine with per-eb stores for a shorter drain chain."""
                state = {}
                act_evac = act_evac or tail

                def start_eb(eb, half):
                    def _q():
                        if half == 0:
                            if tail and eb == 1:
                                pp = scores_pool.tile([128, 512], F32,
                                                      tag="scr", name="pp")
                            else:
                                pp = aux_pool.tile([128, 512], F32,
                                                   tag="aux", name="pp")
                            state[eb] = pp
                            if eb == 0:
                                state["ot"] = out_pool.tile(
                                    [128, E], BF16, tag="ot", name="ot")
                        pp = state[eb]
                        for hpk in (2 * half, 2 * half + 1):
                            nc.tensor.matmul(
                                pp[:, 0:512],
                                lhsT=ctxT[:, hpk, qt_ * 128:(qt_ + 1) * 128],
                                rhs=wo_sb[:, hpk, eb * 512:(eb + 1) * 512],
                                start=(hpk == 0), stop=(hpk == NHP - 1),
                            )
                        if half == 1:
                            ot = state["ot"]
                            esl = slice(eb * 512, (eb + 1) * 512)
                            if act_evac:
                                # split the two evacs across DVE and ACT so
                                # they run concurrently in the drain
                                if eb == 0:
                                    nc.vector.tensor_copy(ot[:, esl],
                                                          pp[:, 0:512])
                                else:
                                    nc.scalar.activation(ot[:, esl],
                                                         pp[:, 0:512], COPY)
                                nc.sync.dma_start(
                                    out_d[qt_ * 128:(qt_ + 1) * 128, esl],
                                    ot[:, esl])
                            else:
                                nc.vector.tensor_copy(ot[:, esl], pp[:, 0:512])
                                if eb == 1:
                                    nc.sync.dma_start(
                                        out_d[qt_ * 128:(qt_ + 1) * 128, :],
                                        ot[:])
                    return _q

                return [start_eb(eb, half) for eb in range(E // 512)
                        for half in (0, 1)]

            def emit_qk(st):
                qb, hp, kk = st
                nfull = (qb + 1) * 4 - 4
                j = kk - nfull
                q0 = max(j, 0) * TK
                q0e = min(q0, 256)
                scr = scores_pool.tile([128, 2, TQ], F32, tag="scr", name="scr")
                boot = (qb == 3 and hp == 0)
                for head in (0, 1):
                    dr = slice(head * D, head * D + D)
                    if boot:
                        lhsT = (boot_sb[dr, 0:TK] if kk == 0
                                else kt_sbs[0][dr, kk * TK:(kk + 1) * TK])
                        rhs = boot_sb[dr, TK + q0e:TK + TQ]
                    else:
                        lhsT = kt_sbs[hp][dr, kk * TK:(kk + 1) * TK]
                        rhs = qt_sbs[hp][dr, qb * TQ + q0e:(qb + 1) * TQ]
                    nc.tensor.matmul(
                        scr[:, head, q0e:TQ],
                        lhsT=lhsT, rhs=rhs,
                        start=True, stop=(j < 0), skip_group_check=True,
                    )
                if j >= 0:
                    # add -1e30 strict-lower-tri mask into the diagonal band
                    for head in (0, 1):
                        nc.tensor.matmul(
                            scr[:, head, q0:q0 + TK],
                            lhsT=i128_sb[:], rhs=lneg_sb[:],
                            start=False, stop=True, skip_group_check=True,
                        )
                return scr, q0

            def emit_exp(scr, q0):
                at = at_pool.tile([128, 2, TQ], BF16, tag="attn", name="at")
                nc.scalar.activation(at[:, :, q0:TQ], scr[:, :, q0:TQ],
                                     EXP, scale=float(SCALE))
                return at

            def emit_pv(st, at, accus):
                qb, hp, kk = st
                nfull = (qb + 1) * 4 - 4
                j = kk - nfull
                for head in (0, 1):
                    h = 2 * hp + head
                    for qcl in range(max(j, 0), 4):
                        # start=True zeroes the whole 2KB PSUM zero-region
                        # (the bank), so only the FIRST matmul into each accu
                        # bank starts; later chunks' first writes land on
                        # pending-zero bytes and overwrite cleanly.
                        nc.tensor.matmul(
                            accus[head][:, qcl, :],
                            lhsT=at[:, head, qcl * TK:(qcl + 1) * TK],
                            rhs=v_sb[:, kk, h, :],
                            start=(kk == 0 and qcl == 0),
                            stop=(kk == nfull + qcl),
                            skip_group_check=True,
                        )

            def make_norm(qb, hp, accus):
                # DVE phase: recips, raw-ctx evac, diag builds. Frees the
                # accu banks for the next head pair's PV.
                def _norm_dve():
                    rc = rc_pool.tile([128, 2, 4], F32, tag="rc", name="rc")
                    cx = ctx_pool.tile([128, 2, 4, D], BF16, tag="cx", name="cx")
                    dgs = []
                    for head in (0, 1):
                        nc.vector.reciprocal(rc[:, head, :],
                                             accus[head][:, :, D])
                        nc.vector.tensor_copy(cx[:, head],
                                              accus[head][:, :, 0:D])
                    for head in (0, 1):
                        for qcl in range(4):
                            dg = diag_pool.tile([128, 128], BF16, tag="diag",
                                                name="dg")
                            nc.vector.tensor_scalar_mul(
                                dg[:], i128_sb[:], rc[:, head, qcl:qcl + 1])
                            dgs.append(dg)
                    return cx, dgs

                # PE phase, as two dispersed quanta (emitted on later steps
                # so the transposes never block the PE queue on the DVE
                # chain above)
                state = {}

                def _norm_pe(head, cx, dgs):
                    if head == 0:
                        state["tp"] = aux_pool.tile([128, TQ], F32,
                                                    tag="aux", name="tp")
                    tp = state["tp"]
                    po = head * D
                    for qcl in range(4):
                        nc.tensor.matmul(
                            tp[po:po + D, qcl * TK:(qcl + 1) * TK],
                            lhsT=cx[:, head, qcl, :],
                            rhs=dgs[head * 4 + qcl][:],
                            start=True, stop=True, skip_group_check=True,
                        )
                    if head == 1:
                        nc.vector.tensor_copy(
                            ctxT[:, hp, qb * TQ:(qb + 1) * TQ], tp[:])
                return _norm_dve, _norm_pe

            steps = []
            for qb, hp in BLOCKS:
                for kk in range((qb + 1) * 4):
                    steps.append((qb, hp, kk))

            def emit_qc_norm_proj(accus, qb, qc):
                """Final-block path: q-chunk qc of (qb, hp=3) is complete;
                normalize+transpose just that chunk and project its q-tile
                while the block's remaining exps still cover the chain."""
                rcq = rc_pool.tile([128, 2], F32, tag="rcq", name="rcq")
                cxq = ctx_pool.tile([128, 2, D], BF16, tag="cxq", name="cxq")
                for head in (0, 1):
                    nc.vector.reciprocal(rcq[:, head:head + 1],
                                         accus[head][:, qc, D:D + 1])
                    nc.vector.tensor_copy(cxq[:, head],
                                          accus[head][:, qc, 0:D])
                tpq = aux_pool.tile([128, TK], F32, tag="aux", name="tpq")
                for head in (0, 1):
                    dg = diag_pool.tile([128, 128], BF16, tag="diag", name="dg")
                    nc.vector.tensor_scalar_mul(dg[:], i128_sb[:],
                                                rcq[:, head:head + 1])
                    nc.tensor.matmul(
                        tpq[head * D:head * D + D, :],
                        lhsT=cxq[:, head, :], rhs=dg[:],
                        start=True, stop=True, skip_group_check=True,
                    )
                dst = ctxT[:, NHP - 1, qb * TQ + qc * TK:qb * TQ + (qc + 1) * TK]
                if qc == 3:
                    # last chunk: DVE may be clogged with proj evacs; ACT is
                    # idle right after the final exp
                    nc.scalar.activation(dst, tpq[:], COPY)
                else:
                    nc.vector.tensor_copy(dst, tpq[:])

            qk_next = emit_qk(steps[0])
            pending_norm = None      # (dve_fn, pe_fn) awaiting DVE phase
            background = []          # dispersed PE work, popped per step
            accus = None
            final_block = steps[-1][:2]   # (qb=0, hp=3)
            for i, st in enumerate(steps):
                qb, hp, kk = st
                nk = (qb + 1) * 4
                scr, q0 = qk_next
                if i + 1 < len(steps):
                    # QKs gate the exps (the critical engine): schedule them
                    # ahead of any concurrently-ready PE work
                    with tc.high_priority(offset=30):
                        qk_next = emit_qk(steps[i + 1])
                if kk == 0:
                    if pending_norm is not None:
                        dve_fn, pe_fn = pending_norm
                        cx, dgs = dve_fn()
                        background.insert(0, lambda pe_fn=pe_fn, cx=cx,
                                          dgs=dgs: pe_fn(1, cx, dgs))
                        background.insert(0, lambda pe_fn=pe_fn, cx=cx,
                                          dgs=dgs: pe_fn(0, cx, dgs))
                        pending_norm = None
                    accus = [accu_pool.tile([128, 4, 65], F32,
                                            tag=f"accu{head}", name="accu")
                             for head in (0, 1)]
                elif background:
                    # adaptive drain: pop harder when the backlog piles up.
                    # Deprioritized so the scheduler slots this work into PE
                    # idle gaps instead of ahead of critical QKs.
                    npop = 1 + (len(background) > 12)
                    with tc.high_priority(offset=-100):
                        for _ in range(min(npop, len(background))):
                            background.pop(0)()
                at = emit_exp(scr, q0)
                emit_pv(st, at, accus)
                nfull = nk - 4
                if (qb, hp) == final_block and kk >= nfull:
                    # per-q-chunk completion: chunk kk-nfull stops here.
                    # The norm chain outranks the deprioritized projections
                    # so the static scheduler never buries it behind them.
                    with tc.high_priority(offset=40):
                        emit_qc_norm_proj(accus, qb, kk - nfull)
                    if kk < nk - 1:
                        with tc.high_priority(offset=-30):
                            for q in proj_quanta(qb * (TQ // 128) + kk - nfull,
                                                 act_evac=(kk > nfull)):
                                q()
                    else:
                        # last tile: the hpk0-2 partial accumulation doesn't
                        # need the final norm; run it under the last steps
                        qt15 = qb * (TQ // 128) + 3
                        pps = []
                        with tc.high_priority(offset=-30):
                            for eb in range(2):
                                if eb == 1:
                                    pp = scores_pool.tile([128, 512], F32,
                                                          tag="scr", name="pp")
                                else:
                                    pp = aux_pool.tile([128, 512], F32,
                                                       tag="aux", name="pp")
                                pps.append(pp)
                                for hpk in range(3):
                                    nc.tensor.matmul(
                                        pp[:, 0:512],
                                        lhsT=ctxT[:, hpk,
                                                  qt15 * 128:(qt15 + 1) * 128],
                                        rhs=wo_sb[:, hpk,
                                                  eb * 512:(eb + 1) * 512],
                                        start=(hpk == 0), stop=False,
                                        skip_group_check=True,
                                    )
                elif kk == nk - 1:
                    pending_norm = make_norm(qb, hp, accus)
                    if hp == NHP - 1:
                        for qt_ in range(qb * (TQ // 128),
                                         (qb + 1) * (TQ // 128)):
                            background.extend(proj_quanta(qt_))
            # last tile: only the hpk3 matmuls + evacs + stores remain; the
            # two evacs go to DVE and ACT so they run concurrently
            ot15 = out_pool.tile([128, E], BF16, tag="ot", name="ot")
            for eb in range(2):
                nc.tensor.matmul(
                    pps[eb][:, 0:512],
                    lhsT=ctxT[:, 3, qt15 * 128:(qt15 + 1) * 128],
                    rhs=wo_sb[:, 3, eb * 512:(eb + 1) * 512],
                    start=False, stop=True, skip_group_check=True,
                )
                esl = slice(eb * 512, (eb + 1) * 512)
                if eb == 0:
                    nc.vector.tensor_copy(ot15[:, esl], pps[eb][:, 0:512])
                else:
                    nc.scalar.activation(ot15[:, esl], pps[eb][:, 0:512], COPY)
                nc.sync.dma_start(out_d[qt15 * 128:(qt15 + 1) * 128, esl],
                                  ot15[:, esl])
            for q in background:
                q()

    nc.compile()
    return nc


_NC_CACHE = {}


def _get_nc():
    if "nc" not in _NC_CACHE:
        _NC_CACHE["nc"] = _build_nc()
    return _NC_CACHE["nc"]


def round_f32r(x):
    """Round fp32 to the float32r grid (11 explicit mantissa bits, RNE)."""
    u = np.ascontiguousarray(x, dtype=np.float32).view(np.uint32)
    r = (u + np.uint32(0x7FF) + ((u >> np.uint32(12)) & np.uint32(1))) & np.uint32(0xFFFFF000)
    return r.view(np.float32)


def build_in_maps(Q, K, V, W_o):
    bf = ml_dtypes.bfloat16
    lneg = np.where(np.arange(TK)[:, None] > np.arange(TK)[None, :],
                    np.float32(-1e30), np.float32(0.0)).astype(bf)
    i128 = np.eye(128, dtype=np.float32).astype(bf)

    in_maps = []
    for c in range(NCORES):
        b, g = c // 2, c % 2
        hs = slice(g * HLOC * D, (g + 1) * HLOC * D)
        qt = np.ascontiguousarray(Q[b][:, hs].T)          # (512, 2048)
        kt = np.ascontiguousarray(K[b][:, hs].T)
        # V natural + ones col: vo[p, kk, h, 0:64] = V_h[kk*128+p], col 64 = 1
        vb = V[b][:, hs].reshape(NKT, 128, HLOC, D)       # (16,128,8,64)
        vo = np.ones((128, NKT, HLOC, D + 1), dtype=np.float32)
        vo[:, :, :, 0:D] = vb.transpose(1, 0, 2, 3)
        wo = np.ascontiguousarray(W_o[hs, :])             # (512, 1024)
        boot = np.concatenate([kt[0:128, 0:TK], qt[0:128, 3 * TQ:4 * TQ]],
                              axis=1)                     # (128, 640)
        in_maps.append({"qt": round_f32r(qt), "kt": round_f32r(kt),
                        "vo": vo.astype(bf), "wo": wo.astype(bf),
                        "lneg": lneg, "i128": i128,
                        "boot": round_f32r(boot)})
    return in_maps


def _kernel_numpy(Q, K, V, mask, W_o, b_o):
    """Reference fallback for non-causal masks (never hit in practice)."""
    out = np.empty((B, T, E), dtype=np.float32)
    for b in range(B):
        q = Q[b].reshape(T, H, D).transpose(1, 0, 2)
        k = K[b].reshape(T, H, D).transpose(1, 0, 2)
        v = V[b].reshape(T, H, D).transpose(1, 0, 2)
        s = np.einsum("hqd,hkd->hqk", q, k) / np.sqrt(D)
        s = np.where(mask[b][None], -np.inf, s)
        a = np.exp(s - s.max(-1, keepdims=True))
        a /= a.sum(-1, keepdims=True)
        ctx = np.einsum("hqk,hkd->hqd", a, v).transpose(1, 0, 2).reshape(T, H * D)
        out[b] = ctx @ W_o + b_o
    return out


_CAUSAL = None


def _is_causal(mask):
    global _CAUSAL
    if _CAUSAL is None:
        _CAUSAL = np.triu(np.ones((T, T), dtype=bool), 1)
    m = np.asarray(mask)
    return m.shape == (B, T, T) and all(np.array_equal(m[b], _CAUSAL) for b in range(B))


def kernel(Q, K, V, mask, W_o, b_o):
    Q = np.asarray(Q, dtype=np.float32)
    K = np.asarray(K, dtype=np.float32)
    V = np.asarray(V, dtype=np.float32)
    W_o = np.asarray(W_o, dtype=np.float32)
    b_o = np.asarray(b_o, dtype=np.float32)

    if not _is_causal(mask):
        return _kernel_numpy(Q, K, V, np.asarray(mask, dtype=bool), W_o, b_o)

    in_maps = build_in_maps(Q, K, V, W_o)

    nc = _get_nc()
    res = run_bass_kernel_spmd(nc, in_maps, core_ids=list(range(NCORES)))
    _NC_CACHE["last_results"] = res

    out = np.empty((B, T, E), dtype=np.float32)
    for b in range(B):
        out[b] = (np.asarray(res.results[2 * b]["out"], dtype=np.float32)
                  + np.asarray(res.results[2 * b + 1]["out"], dtype=np.float32))
    out += b_o
    return out
